# revision 52
# baseline (speedup 1.0000x reference)
"""Trainium2 Bass kernel for nn_Attention_75093208203309 (sparse attention).

Contract: kernel(**inputs) takes FULL unsharded inputs (numpy), returns the
FULL [4096, 1024] float32 output. Internally shards query rows across 8
NeuronCores.

Strategy (host-side weight folding + minimal collectives):
  With S-tiles oriented [k-rows, own-rows], both logit matrices factor
  through x directly:
    B = k @ q^T  = x_j (Wk^T Wq)        x_o^T + (x_j Wk^T b_q) 1^T + 1 rowB + s
    A = k @ qc^T = x_j (Wk^T C^T Wq)    x_o^T + (x_j Wk^T C^T b_q) 1^T
                   + 1 (x_o (C b_k @ Wq)^T + b_q C b_k)^T
  M1 = Wk^T Wq and M2 = Wk^T C^T Wq are folded on the host (bf16), so the
  device needs NO q/qc/k projections at all: per own-row block we compute
  PB = M1^T-chunks @ x_o^T and PA likewise (two 27us projections), then the
  S matmuls use replicated bf16 x^T blocks (xtf input) as the stationary
  operand. B's own-row bias term is constant within each softmax row and
  cancels — dropped. The remaining bias terms are added with K=1 rank-1
  matmuls into the PSUM accumulation groups.
  v is computed for the core's own rows (f32r) and all-gathered ONCE (bf16),
  issued ~45us in and consumed by the O phase ~250us in — fully hidden.
  The vt loads ride the gpsimd queue (they depend on the collective; on the
  in-order SP queue they would head-of-line-block the S-phase streams).
  uint8 masks ride one packed [128, 1024] DMA per g-tile (am | lm).
  exp uses a fixed -20000 shift (class-2 mask entries dominate every row);
  softmax normalization via per-partition reciprocal sums after O = E @ v.
"""

import contextlib

import numpy as np
import ml_dtypes  # noqa: F401  (np bfloat16 views)

import concourse.bass as bass
import concourse.bacc as bacc
import concourse.mybir as mybir
import concourse.tile as tile
from concourse import bass_utils

f32 = mybir.dt.float32
f32r = mybir.dt.float32r
bf16 = mybir.dt.bfloat16
AF = mybir.ActivationFunctionType
ALU = mybir.AluOpType

NCORES = 8
N, D = 4096, 1024
M = N // NCORES          # 512 rows per core
MT = M // 128            # 4 m-tiles
G = N // 128             # 32 mk-tiles
DC = D // 128            # 8 contraction tiles
MSCALE = 320000.0        # 10000 * 32 (folds softmax scale 1/sqrt(D)=1/32)
RG = [list(range(NCORES))]


def build(bias_val: float, timing_mode: bool = False, repeats: int = 1,
          serial: bool = False):
    """timing_mode: single-core variant with zv as ExternalInput and no
    collectives, for TimelineSim cost-model profiling.
    serial: share v_loc across repeats so reps cannot pipeline through the
    collective — an R-unroll proxy for exposed single-shot latency."""
    nc = bacc.Bacc(None, num_devices=NCORES, debug=False)

    xt = nc.dram_tensor("xt", [DC, 128, M], f32, kind="ExternalInput")
    xn = nc.dram_tensor("xn", [MT, 128, D], f32, kind="ExternalInput")
    xbo = nc.dram_tensor("xbo", [DC, 128, M], bf16, kind="ExternalInput")
    xtf = nc.dram_tensor("xtf", [NCORES, DC, 128, M], bf16, kind="ExternalInput")
    m1t = nc.dram_tensor("m1t", [DC, 128, D], bf16, kind="ExternalInput")
    m2t = nc.dram_tensor("m2t", [DC, 128, D], bf16, kind="ExternalInput")
    cc2 = nc.dram_tensor("cc2", [DC, 128, 2], bf16, kind="ExternalInput")
    wra = nc.dram_tensor("wra", [DC, 128, 1], bf16, kind="ExternalInput")
    sa1 = nc.dram_tensor("sa1", [1, 1], f32, kind="ExternalInput")
    wvt = nc.dram_tensor("wvt", [DC, 128, D], f32, kind="ExternalInput")
    bv = nc.dram_tensor("bv", [1, D], f32, kind="ExternalInput")
    bnd = nc.dram_tensor("bnd", [MT, 128, 1], f32, kind="ExternalInput")
    mk = nc.dram_tensor("mk", [G, 128, 2 * M], mybir.dt.uint8, kind="ExternalInput")
    ones8 = nc.dram_tensor("ones8", [128, 8], mybir.dt.bfloat16, kind="ExternalInput")
    ones1 = nc.dram_tensor("ones1", [1, 128], f32, kind="ExternalInput")
    onesb = nc.dram_tensor("onesb", [1, M], bf16, kind="ExternalInput")
    out = nc.dram_tensor("out", [MT, 128, D], f32, kind="ExternalOutput")

    with tile.TileContext(nc) as tc, contextlib.ExitStack() as ST:
        pp = ST.enter_context(tc.tile_pool(name="persist", bufs=1))
        dp = ST.enter_context(tc.tile_pool(name="dram", bufs=1, space="DRAM"))

        ones_s = pp.tile([128, 8], bf16, name="ones_s")
        onesk1 = pp.tile([1, 128], f32r, name="onesk1")
        onesb_s = pp.tile([1, M], bf16, name="onesb_s")
        onesm_s = pp.tile([1, 128], bf16, name="onesm_s")
        bv_s = pp.tile([1, D], f32r, name="bv_s")
        sa_s = pp.tile([1, 1], f32, name="sa_s")
        rowa_s = pp.tile([1, M], bf16, name="rowa_s")
        bnd_s = pp.tile([128, MT], f32, name="bnd_s")
        recip_s = pp.tile([128, MT], f32, name="recip_s")
        s1_s = pp.tile([128, MT], f32, name="s1_s")
        omb_s = pp.tile([128, MT], f32, name="omb_s")
        shift_s = pp.tile([128, 1], f32, name="shift_s")
        nc.vector.memset(shift_s[:], -20000.0)

        nc.sync.dma_start(ones_s[:], ones8.ap())
        nc.sync.dma_start(onesk1[:], ones1.ap().bitcast(f32r))
        nc.sync.dma_start(onesb_s[:], onesb.ap())
        nc.sync.dma_start(onesm_s[:], onesb.ap()[:, 0:128])
        nc.sync.dma_start(bv_s[:], bv.ap().bitcast(f32r))
        nc.sync.dma_start(sa_s[:], sa1.ap())
        nc.sync.dma_start(bnd_s[:], bnd.ap().rearrange("m p one -> p (m one)"))
        nc.vector.tensor_scalar(omb_s[:], bnd_s[:], -1.0, 1.0, ALU.mult, ALU.add)

        if timing_mode:
            zv = [
                nc.dram_tensor(f"zv{h}", [NCORES, MT, 128, 512], bf16,
                               kind="ExternalInput").ap()
                for h in range(2)
            ]
        elif serial:
            v_loc = [
                dp.tile([MT, 128, 512], bf16, name=f"v_loc{h}") for h in range(2)
            ]

        for _rep in range(repeats):
            if not timing_mode:
                zv = [
                    dp.tile([NCORES, MT, 128, 512], bf16, name=f"zv{h}_{_rep}",
                            addr_space="Shared")
                    for h in range(2)
                ]
            if not serial:
                v_loc = [
                    dp.tile([MT, 128, 512], bf16, name=f"v_loc{h}_{_rep}")
                    for h in range(2)
                ]
            E3 = [
                pp.tile([128, M], bf16, tag="E3", name=f"E3_{g}_{_rep}", bufs=G)
                for g in range(G)
            ]
            # pools whose lifetimes cross phase boundaries, closed manually
            q_stack = contextlib.ExitStack()
            qp = q_stack.enter_context(tc.tile_pool(name="qpool", bufs=1))
            pb_s = qp.tile([128, DC, M], bf16, name="pb_s")
            pa_s = qp.tile([128, DC, M], bf16, name="pa_s")
            xbo_s = qp.tile([128, DC, M], bf16, name="xbo_s")
            cc2_s = qp.tile([128, DC, 2], bf16, name="cc2_s")
            wra_s = qp.tile([128, DC, 1], bf16, name="wra_s")

            # ------------- v projection + folded PB/PA projections -------------
            with (
                tc.tile_pool(name="qkv_w", bufs=3) as wp,
                tc.tile_pool(name="qkv_x", bufs=1) as xp,
                tc.tile_pool(name="qkv_sb", bufs=3) as sp,
                tc.tile_pool(name="qkv_ps", bufs=8, space="PSUM") as ps1,
            ):
                xt_s = xp.tile([128, DC, M], f32r, name="xt_s")

                def load_xt_half(th):
                    tsl = slice(th * 4, (th + 1) * 4)
                    nc.sync.dma_start(
                        xt_s[:, tsl, :],
                        xt.ap()[tsl].rearrange("t p m -> p t m").bitcast(f32r),
                    )

                def load_w_half(wdram, half, name, dt=f32r, tag="w", bufs=None,
                                ths=((0, 4), (4, 8))):
                    kw = {"bufs": bufs} if bufs else {}
                    w_h = wp.tile([128, DC, 512], dt, tag=tag,
                                  name=f"w_{name}{half}", **kw)
                    for lo, hi in ths:
                        tsl = slice(lo, hi)
                        src = (wdram.ap()[tsl].rearrange("t p d -> p t d")
                               [:, :, half * 512 : (half + 1) * 512])
                        if dt == f32r:
                            src = src.bitcast(f32r)
                        nc.sync.dma_start(w_h[:, tsl, :], src)
                    return w_h

                # v first: it feeds the chunked all-gathers. Load order is
                # the consumption order of the first v matmul group (mt-outer,
                # t inner): xt t0-3, wv half0 t0-3, xt t4-7, wv half0 t4-7.
                # The folded mats follow (needed ~80us in), ahead of
                # xbo/cc2/wra (needed later).
                load_xt_half(0)
                wv_h = [load_w_half(wvt, 0, "v")]
                load_xt_half(1)
                wv_h.append(load_w_half(wvt, 1, "v"))
                nc.sync.dma_start(
                    xbo_s[:], xbo.ap().rearrange("t p m -> p t m")
                )
                nc.sync.dma_start(
                    cc2_s[:], cc2.ap().rearrange("t p c -> p t c")
                )
                nc.sync.dma_start(
                    wra_s[:], wra.ap().rearrange("t p c -> p t c")
                )
                m1_h = [load_w_half(m1t, h, "m1", dt=bf16, tag="wb", bufs=4)
                        for h in range(2)]
                m2_h = [load_w_half(m2t, h, "m2", dt=bf16, tag="wb", bufs=4)
                        for h in range(2)]
                # per d-half: project, bias, store, gather — chunked so the
                # dh=0 gather is in flight while dh=1 is still projecting.
                # Stores + collectives ride the gpsimd queue so they never
                # block the SP weight-load stream.
                for dh in range(2):
                    for mt in range(MT):
                        vps = ps1.tile(
                            [128, 512], f32, tag="ps1", name=f"vps{dh}{mt}"
                        )
                        for t in range(DC):
                            nc.tensor.matmul(
                                vps[:],
                                xt_s[:, t, mt * 128 : (mt + 1) * 128],
                                wv_h[dh][:, t, :],
                                start=(t == 0),
                                stop=False,
                            )
                        # high priority: the gathers (and thus the whole
                        # collective pipeline) hang off the stores
                        with tc.high_priority():
                            nc.tensor.matmul(
                                vps[:],
                                onesk1[:, :],
                                bv_s[:, dh * 512 : (dh + 1) * 512],
                                start=False,
                                stop=True,
                            )
                            v_sb = sp.tile([128, 512], bf16, tag="vsb",
                                           name="v_sb", bufs=8)
                            # DVE copy: keeps the chain off the ACT queue's
                            # table-load startup
                            nc.vector.tensor_scalar(
                                v_sb[:], vps[:], 1.0, None, ALU.mult
                            )
                            nc.gpsimd.dma_start(v_loc[dh][mt], v_sb[:])
                    if not timing_mode:
                        nc.gpsimd.collective_compute(
                            "AllGather", ALU.bypass, replica_groups=RG,
                            ins=[v_loc[dh][:].opt()], outs=[zv[dh][:].opt()],
                        )

                # rowA = x_o @ wrA + sA  (one [1,512] strip)
                rps = ps1.tile([1, M], f32, tag="ps1", name="rps")
                for t in range(DC):
                    nc.tensor.matmul(
                        rps[:],
                        wra_s[:, t, :],
                        xbo_s[:, t, :],
                        start=(t == 0),
                        stop=(t == DC - 1),
                    )
                nc.scalar.activation(
                    rowa_s[:], rps[:], AF.Identity, bias=sa_s[:, 0:1]
                )

                # PB = M1^T-chunks @ x_o^T ; PA = M2^T-chunks @ x_o^T
                def fold_proj(m_h, dst):
                    for half in range(2):
                        fps = [
                            ps1.tile([128, M], f32, tag="ps1", name="fps")
                            for _ in range(4)
                        ]
                        for t in range(DC):
                            for oi in range(4):
                                nc.tensor.matmul(
                                    fps[oi][:],
                                    m_h[half][:, t, oi * 128 : (oi + 1) * 128],
                                    xbo_s[:, t, :],
                                    start=(t == 0),
                                    stop=(t == DC - 1),
                                )
                        for oi in range(4):
                            ot = half * 4 + oi
                            nc.scalar.copy(dst[:, ot, :], fps[oi][:])

                fold_proj(m1_h, pb_s)
                fold_proj(m2_h, pa_s)

            # v tiles + xn survive into the O phase
            o_stack = contextlib.ExitStack()
            vpool = o_stack.enter_context(
                tc.tile_pool(name="o_v", bufs=3, side="right")
            )
            xop = o_stack.enter_context(
                tc.tile_pool(name="o_x", bufs=1, side="right")
            )
            xn_s = xop.tile([128, MT, D], f32, name="xn_s")
            vt_pre = {}

            def load_vt(dh, j):
                vt = vpool.tile([128, 4, 512], bf16, tag="v", name="vt", bufs=4)
                for vb in range(4):
                    # gpsimd queue: these loads wait on the collective, and on
                    # the in-order SP queue they would head-of-line-block the
                    # S phase's mask/xtb streams
                    nc.gpsimd.dma_start(vt[:, vb, :], zv[dh][j][vb])
                vt_pre[(dh, j)] = vt
                return vt

            # ------- S phase: logits via folded mats, mask, exp -------
            with (
                tc.tile_pool(name="s_x", bufs=3) as xbp,
                tc.tile_pool(name="s_kc", bufs=3) as kcp,
                tc.tile_pool(name="s_m", bufs=6) as mp,
                tc.tile_pool(name="s_t", bufs=4) as tpool,
                tc.tile_pool(name="s_psK", bufs=2, space="PSUM") as psK,
                tc.tile_pool(name="s_psA", bufs=3, space="PSUM") as psA,
                tc.tile_pool(name="s_psB", bufs=3, space="PSUM") as psB,
            ):
                xtb_pre = {}

                def load_xtb(j, eng=None):
                    xtb = xbp.tile([128, DC, M], bf16, tag="xtb", name="xtb")
                    (eng or nc.sync).dma_start(
                        xtb[:], xtf.ap()[j].rearrange("t p m -> p t m")
                    )
                    xtb_pre[j] = xtb
                    return xtb

                kc_pre = {}
                mk_pre = {}

                def load_mk(g):
                    mk_t = mp.tile([128, 2 * M], mybir.dt.uint8, tag="mk",
                                   name="mk_t")
                    nc.sync.dma_start(mk_t[:], mk.ap()[g])
                    mk_pre[g] = mk_t
                    return mk_t

                for g0 in range(4):
                    load_mk(g0)

                def compute_kc(j, xtb):
                    # separate [1, M] groups: engines cannot address a
                    # partition range starting at 1, so a [2, M] strip's
                    # second row would be unreadable
                    outs = []
                    for ci, tag in ((0, "kcb"), (1, "kca")):
                        kps = psK.tile([1, M], f32, tag="kc", name=f"kcps{ci}")
                        for t in range(DC):
                            nc.tensor.matmul(
                                kps[:],
                                cc2_s[:, t, ci : ci + 1],
                                xtb[:, t, :],
                                start=(t == 0),
                                stop=(t == DC - 1),
                            )
                        kc = kcp.tile([1, M], bf16, tag=tag, name=tag)
                        nc.scalar.copy(kc[:], kps[:])
                        outs.append(kc)
                    kc_pre[j] = tuple(outs)

                load_xtb(0)
                for j in range(NCORES):
                    xtb = xtb_pre.pop(j)
                    if j + 1 < NCORES:
                        # ACT queue: dispatch is paced by the S phase's exp
                        # stream, so these 2MB transfers stay out of the DMA
                        # window the v stores + gathers need
                        load_xtb(j + 1, eng=nc.scalar)
                    compute_kc(j, xtb)
                    kcb, kca = kc_pre.pop(j)
                    for gi in range(4):
                        g = j * 4 + gi
                        B = psB.tile([128, M], f32, tag="B", name="Bps")
                        for t in range(DC):
                            nc.tensor.matmul(
                                B[:],
                                xtb[:, t, gi * 128 : (gi + 1) * 128],
                                pb_s[:, t, :],
                                start=(t == 0),
                                stop=False,
                            )
                        nc.tensor.matmul(
                            B[:], kcb[:, gi * 128 : (gi + 1) * 128],
                            onesb_s[:], start=False, stop=True,
                        )
                        A = psA.tile([128, M], f32, tag="A", name="Aps")
                        for t in range(DC):
                            nc.tensor.matmul(
                                A[:],
                                xtb[:, t, gi * 128 : (gi + 1) * 128],
                                pa_s[:, t, :],
                                start=(t == 0),
                                stop=False,
                            )
                        nc.tensor.matmul(
                            A[:], kca[:, gi * 128 : (gi + 1) * 128],
                            onesb_s[:], start=False, stop=False,
                        )
                        nc.tensor.matmul(
                            A[:], onesm_s[:], rowa_s[:], start=False, stop=True,
                        )
                        if g + 4 < G:
                            load_mk(g + 4)
                        mk_t = mk_pre.pop(g)
                        t3 = tpool.tile([128, M], f32, tag="t3", name="t3")
                        nc.vector.scalar_tensor_tensor(
                            t3[:], A[:], -bias_val, mk_t[:, M : 2 * M],
                            ALU.is_gt, ALU.mult,
                        )
                        nc.vector.tensor_tensor(
                            t3[:], t3[:], mk_t[:, 0:M], ALU.add
                        )
                        comb = tpool.tile([128, M], f32, tag="comb", name="comb")
                        nc.vector.scalar_tensor_tensor(
                            comb[:], t3[:], MSCALE, B[:], ALU.mult, ALU.add
                        )
                        # -20000 = the (am+st*lm-2) shift, folded into the exp bias
                        nc.scalar.activation(
                            E3[g][:], comb[:], AF.Exp, scale=1.0 / 32.0,
                            bias=shift_s[:, 0:1],
                        )
                    if j == 1:
                        # xn is consumed only by the O phase; emitting the
                        # load here keeps its 2MB transfer out of the early
                        # DMA window that the v stores + gathers need
                        nc.sync.dma_start(
                            xn_s[:], xn.ap().rearrange("m p d -> p m d")
                        )
            q_stack.close()  # PB/PA/xbo SBUF released before O phase

            # ---------------- O phase: E @ v, sums, blend ----------------
            with (
                tc.tile_pool(name="o_out", bufs=4) as opool,
                tc.tile_pool(name="o_xm", bufs=1) as xmp,
                tc.tile_pool(name="o_ps", bufs=1, space="PSUM") as psO,
            ):
                # xm = (1-boundary)*x for all 8 chunks, off the critical path
                xm_t = {}
                for mt in range(MT):
                    for dh in range(2):
                        xm = xmp.tile([128, 512], f32, name=f"xm_{mt}_{dh}")
                        nc.vector.tensor_scalar(
                            xm[:],
                            xn_s[:, mt, dh * 512 : (dh + 1) * 512],
                            omb_s[:, mt : mt + 1],
                            None,
                            ALU.mult,
                        )
                        xm_t[(mt, dh)] = xm
                # Ssum first: needs no v, so it runs while gathers are still
                # in flight and buys the dh=1 gather a little more slack
                S_ps = [
                    psO.tile([128, 8], f32, tag="Ssum", name=f"S{mt}", bufs=4)
                    for mt in range(MT)
                ]
                for g in range(G):
                    for mt in range(MT):
                        nc.tensor.matmul(
                            S_ps[mt][:],
                            E3[g][:, mt * 128 : (mt + 1) * 128],
                            ones_s[:],
                            start=(g == 0),
                            stop=(g == G - 1),
                        )
                for mt in range(MT):
                    nc.vector.reciprocal(
                        recip_s[:, mt : mt + 1], S_ps[mt][:, 0:1]
                    )
                    nc.vector.tensor_tensor(
                        s1_s[:, mt : mt + 1],
                        recip_s[:, mt : mt + 1],
                        bnd_s[:, mt : mt + 1],
                        ALU.mult,
                    )
                for dh in range(2):
                    O_ps = [
                        psO.tile([128, 512], f32, tag="O", name=f"O{dh}_{mt}", bufs=4)
                        for mt in range(MT)
                    ]
                    for j in range(NCORES):
                        vt = vt_pre.pop((dh, j), None) or load_vt(dh, j)
                        vt_pre.pop((dh, j), None)
                        for gi in range(4):
                            g = j * 4 + gi
                            for mt in range(MT):
                                nc.tensor.matmul(
                                    O_ps[mt][:],
                                    E3[g][:, mt * 128 : (mt + 1) * 128],
                                    vt[:, gi, :],
                                    start=(g == 0),
                                    stop=(g == G - 1),
                                )
                    for mt in range(MT):
                        # out = O*(boundary/sumexp) + (1-boundary)*x, one DVE op
                        ot_t = opool.tile([128, 512], f32, tag="ot", name="ot_t")
                        nc.vector.scalar_tensor_tensor(
                            ot_t[:], O_ps[mt][:], s1_s[:, mt : mt + 1],
                            xm_t[(mt, dh)][:], ALU.mult, ALU.add,
                        )
                        nc.sync.dma_start(
                            out.ap()[mt, :, dh * 512 : (dh + 1) * 512], ot_t[:]
                        )
            o_stack.close()

    nc.compile()
    return nc


def make_in_maps(x, attention_mask, learnable_mask, boundary_mask,
                 W_q, b_q, W_k, b_k, W_v, b_v, connection):
    bf = ml_dtypes.bfloat16
    x = np.asarray(x, np.float32)
    W_q = np.asarray(W_q, np.float32)
    W_k = np.asarray(W_k, np.float32)
    W_v = np.asarray(W_v, np.float32)
    C = np.asarray(connection, np.float32)
    b_q = np.asarray(b_q, np.float32)
    b_k = np.asarray(b_k, np.float32)
    amh_full = np.asarray(attention_mask, np.float32).astype(np.uint8)
    lmh_full = np.asarray(learnable_mask, np.float32).astype(np.uint8)
    boundary = np.asarray(boundary_mask, np.float32).reshape(N)

    # Folded matrices (f32 accumulate, bf16 ship)
    M1 = W_k.T @ W_q                       # B main term
    M2 = W_k.T @ C.T @ W_q                 # A main term
    c1 = W_k.T @ b_q                       # B per-k-row column
    c2 = W_k.T @ (C.T @ b_q)               # A per-k-row column
    wrA = (C @ b_k) @ W_q                  # A per-own-row vector
    sA = float(b_q @ (C @ b_k))            # A scalar

    xtf_h = np.ascontiguousarray(
        np.stack([x[j * M : (j + 1) * M].T.astype(bf) for j in range(NCORES)])
    ).reshape(NCORES, DC, 128, M)
    m1t_h = np.ascontiguousarray(M1.T.astype(bf)).reshape(DC, 128, D)
    m2t_h = np.ascontiguousarray(M2.T.astype(bf)).reshape(DC, 128, D)
    cc2_h = np.ascontiguousarray(
        np.stack([c1, c2], axis=1).astype(bf)
    ).reshape(DC, 128, 2)
    wra_h = np.ascontiguousarray(wrA.astype(bf)).reshape(DC, 128, 1)
    wvt_h = np.ascontiguousarray(W_v.T).reshape(DC, 128, D)
    bv_h = np.ascontiguousarray(np.asarray(b_v, np.float32).reshape(1, D))
    in_maps = []
    for c in range(NCORES):
        rows = slice(c * M, (c + 1) * M)
        # packed masks: per g-tile [128 k-rows, own 512 | own 512]
        amT = np.ascontiguousarray(amh_full[rows].T).reshape(G, 128, M)
        lmT = np.ascontiguousarray(lmh_full[rows].T).reshape(G, 128, M)
        mk_h = np.ascontiguousarray(np.concatenate([amT, lmT], axis=2))
        in_maps.append(dict(
            xt=np.ascontiguousarray(x[rows].T).reshape(DC, 128, M),
            xn=np.ascontiguousarray(x[rows]).reshape(MT, 128, D),
            xbo=np.ascontiguousarray(x[rows].T.astype(bf)).reshape(DC, 128, M),
            xtf=xtf_h,
            m1t=m1t_h, m2t=m2t_h, cc2=cc2_h, wra=wra_h,
            sa1=np.full((1, 1), sA, np.float32),
            wvt=wvt_h, bv=bv_h,
            bnd=np.ascontiguousarray(boundary[rows]).reshape(MT, 128, 1),
            mk=mk_h,
            ones8=np.ones((128, 8), dtype=bf),
            ones1=np.ones((1, 128), dtype=np.float32),
            onesb=np.ones((1, M), dtype=bf),
        ))
    return in_maps


_cache = {}


def kernel(x, attention_mask, learnable_mask, boundary_mask,
           W_q, b_q, W_k, b_k, W_v, b_v, connection, bias):
    bias_val = float(np.asarray(bias).reshape(-1)[0])
    if bias_val not in _cache:
        _cache[bias_val] = build(bias_val)
    nc = _cache[bias_val]
    in_maps = make_in_maps(x, attention_mask, learnable_mask, boundary_mask,
                           W_q, b_q, W_k, b_k, W_v, b_v, connection)
    res = bass_utils.run_bass_kernel_spmd(nc, in_maps, core_ids=list(range(NCORES)))
    outs = [res.results[c]["out"].reshape(M, D) for c in range(NCORES)]
    return np.concatenate(outs, axis=0).astype(np.float32)


# revision 55
# speedup vs baseline: 1.3646x; 1.3646x over previous
"""Trainium2 Bass kernel for nn_Attention_75093208203309 (sparse attention).

Contract: kernel(**inputs) takes FULL unsharded inputs (numpy), returns the
FULL [4096, 1024] float32 output. Internally shards query rows across 8
NeuronCores.

Strategy (host-side weight folding + minimal collectives):
  With S-tiles oriented [k-rows, own-rows], both logit matrices factor
  through x directly:
    B = k @ q^T  = x_j (Wk^T Wq)        x_o^T + (x_j Wk^T b_q) 1^T + 1 rowB + s
    A = k @ qc^T = x_j (Wk^T C^T Wq)    x_o^T + (x_j Wk^T C^T b_q) 1^T
                   + 1 (x_o (C b_k @ Wq)^T + b_q C b_k)^T
  M1 = Wk^T Wq and M2 = Wk^T C^T Wq are folded on the host (bf16), so the
  device needs NO q/qc/k projections at all: per own-row block we compute
  PB = M1^T-chunks @ x_o^T and PA likewise (two 27us projections), then the
  S matmuls use replicated bf16 x^T blocks (xtf input) as the stationary
  operand. B's own-row bias term is constant within each softmax row and
  cancels — dropped. The remaining bias terms are added with K=1 rank-1
  matmuls into the PSUM accumulation groups.
  v is computed for the core's own rows (f32r) and all-gathered in two
  d-half chunks (bf16, ~37us + ~158us issue) so the O phase's first half
  never waits; only the second half's tail is exposed. The v store chain is
  high-priority and the vt loads ride the gpsimd queue (they depend on the
  collective; on the in-order SP queue they would head-of-line-block the
  S-phase streams). The xtf block loads ride the ACT queue, paced by the
  exp stream, keeping the early DMA window free for the v stores.
  uint8 masks ride one packed [128, 1024] DMA per g-tile (am | lm).
  exp uses a fixed -20000 shift (class-2 mask entries dominate every row);
  softmax normalization via per-partition reciprocal sums after O = E @ v.
"""

import contextlib

import numpy as np
import ml_dtypes  # noqa: F401  (np bfloat16 views)

import concourse.bass as bass
import concourse.bacc as bacc
import concourse.mybir as mybir
import concourse.tile as tile
from concourse import bass_utils

f32 = mybir.dt.float32
f32r = mybir.dt.float32r
bf16 = mybir.dt.bfloat16
AF = mybir.ActivationFunctionType
ALU = mybir.AluOpType

NCORES = 8
N, D = 4096, 1024
M = N // NCORES          # 512 rows per core
MT = M // 128            # 4 m-tiles
G = N // 128             # 32 mk-tiles
DC = D // 128            # 8 contraction tiles
MSCALE = 320000.0        # 10000 * 32 (folds softmax scale 1/sqrt(D)=1/32)
RG = [list(range(NCORES))]


def build(bias_val: float, timing_mode: bool = False, repeats: int = 1,
          serial: bool = False):
    """timing_mode: single-core variant with zv as ExternalInput and no
    collectives, for TimelineSim cost-model profiling.
    serial: share v_loc across repeats so reps cannot pipeline through the
    collective — an R-unroll proxy for exposed single-shot latency."""
    nc = bacc.Bacc(None, num_devices=NCORES, debug=False)

    xn = nc.dram_tensor("xn", [MT, 128, D], f32, kind="ExternalInput")
    xbo = nc.dram_tensor("xbo", [DC, 128, M], bf16, kind="ExternalInput")
    xtf = nc.dram_tensor("xtf", [NCORES, DC, 128, M], bf16, kind="ExternalInput")
    m1t = nc.dram_tensor("m1t", [DC, 128, D], bf16, kind="ExternalInput")
    m2t = nc.dram_tensor("m2t", [DC, 128, D], bf16, kind="ExternalInput")
    cc2 = nc.dram_tensor("cc2", [DC, 128, 2], bf16, kind="ExternalInput")
    wra = nc.dram_tensor("wra", [DC, 128, 1], bf16, kind="ExternalInput")
    sa1 = nc.dram_tensor("sa1", [1, 1], f32, kind="ExternalInput")
    wvt = nc.dram_tensor("wvt", [DC, 128, D], bf16, kind="ExternalInput")
    bvb = nc.dram_tensor("bvb", [1, D], bf16, kind="ExternalInput")
    bnd = nc.dram_tensor("bnd", [MT, 128, 1], f32, kind="ExternalInput")
    mk = nc.dram_tensor("mk", [G, 128, 2 * M], mybir.dt.uint8, kind="ExternalInput")
    ones8 = nc.dram_tensor("ones8", [128, 8], mybir.dt.bfloat16, kind="ExternalInput")
    ones1 = nc.dram_tensor("ones1", [1, 128], f32, kind="ExternalInput")
    onesb = nc.dram_tensor("onesb", [1, M], bf16, kind="ExternalInput")
    out = nc.dram_tensor("out", [MT, 128, D], f32, kind="ExternalOutput")

    with tile.TileContext(nc) as tc, contextlib.ExitStack() as ST:
        pp = ST.enter_context(tc.tile_pool(name="persist", bufs=1))
        dp = ST.enter_context(tc.tile_pool(name="dram", bufs=1, space="DRAM"))

        ones_s = pp.tile([128, 8], bf16, name="ones_s")
        onesk1 = pp.tile([1, 128], f32r, name="onesk1")
        onesb_s = pp.tile([1, M], bf16, name="onesb_s")
        onesm_s = pp.tile([1, 128], bf16, name="onesm_s")
        bv_s = pp.tile([1, D], bf16, name="bv_s")
        sa_s = pp.tile([1, 1], f32, name="sa_s")
        rowa_s = pp.tile([1, M], bf16, name="rowa_s")
        bnd_s = pp.tile([128, MT], f32, name="bnd_s")
        recip_s = pp.tile([128, MT], f32, name="recip_s")
        s1_s = pp.tile([128, MT], f32, name="s1_s")
        omb_s = pp.tile([128, MT], f32, name="omb_s")
        shift_s = pp.tile([128, 1], f32, name="shift_s")
        nc.vector.memset(shift_s[:], -20000.0)

        nc.sync.dma_start(ones_s[:], ones8.ap())
        nc.sync.dma_start(onesk1[:], ones1.ap().bitcast(f32r))
        nc.sync.dma_start(onesb_s[:], onesb.ap())
        nc.sync.dma_start(onesm_s[:], onesb.ap()[:, 0:128])
        nc.sync.dma_start(bv_s[:], bvb.ap())
        nc.sync.dma_start(sa_s[:], sa1.ap())
        nc.sync.dma_start(bnd_s[:], bnd.ap().rearrange("m p one -> p (m one)"))
        nc.vector.tensor_scalar(omb_s[:], bnd_s[:], -1.0, 1.0, ALU.mult, ALU.add)

        if timing_mode:
            zv = [
                nc.dram_tensor(f"zv{h}", [NCORES, MT, 128, 512], bf16,
                               kind="ExternalInput").ap()
                for h in range(2)
            ]
        elif serial:
            v_loc = [
                dp.tile([MT, 128, 512], bf16, name=f"v_loc{h}") for h in range(2)
            ]

        for _rep in range(repeats):
            if not timing_mode:
                zv = [
                    dp.tile([NCORES, MT, 128, 512], bf16, name=f"zv{h}_{_rep}",
                            addr_space="Shared")
                    for h in range(2)
                ]
            if not serial:
                v_loc = [
                    dp.tile([MT, 128, 512], bf16, name=f"v_loc{h}_{_rep}")
                    for h in range(2)
                ]
            E3 = [
                pp.tile([128, M], bf16, tag="E3", name=f"E3_{g}_{_rep}", bufs=G)
                for g in range(G)
            ]
            # pools whose lifetimes cross phase boundaries, closed manually
            q_stack = contextlib.ExitStack()
            qp = q_stack.enter_context(tc.tile_pool(name="qpool", bufs=1))
            pb_s = qp.tile([128, DC, M], bf16, name="pb_s")
            pa_s = qp.tile([128, DC, M], bf16, name="pa_s")
            xbo_s = qp.tile([128, DC, M], bf16, name="xbo_s")
            cc2_s = qp.tile([128, DC, 2], bf16, name="cc2_s")
            wra_s = qp.tile([128, DC, 1], bf16, name="wra_s")

            # ------------- v projection + folded PB/PA projections -------------
            with (
                tc.tile_pool(name="qkv_w", bufs=3) as wp,
                tc.tile_pool(name="qkv_x", bufs=1) as xp,
                tc.tile_pool(name="qkv_sb", bufs=3) as sp,
                tc.tile_pool(name="qkv_ps", bufs=8, space="PSUM") as ps1,
            ):

                def load_w_half(wdram, half, name, dt=f32r, tag="w", bufs=None,
                                ths=((0, 4), (4, 8))):
                    kw = {"bufs": bufs} if bufs else {}
                    w_h = wp.tile([128, DC, 512], dt, tag=tag,
                                  name=f"w_{name}{half}", **kw)
                    for lo, hi in ths:
                        tsl = slice(lo, hi)
                        src = (wdram.ap()[tsl].rearrange("t p d -> p t d")
                               [:, :, half * 512 : (half + 1) * 512])
                        if dt == f32r:
                            src = src.bitcast(f32r)
                        nc.sync.dma_start(w_h[:, tsl, :], src)
                    return w_h

                # v first: it feeds the chunked all-gathers. The whole v
                # projection is bf16 (xbo x wv-bf16) so the critical-window
                # input DMA is minimal and the stores reach the DMA engines
                # early. The folded mats follow (needed ~45us in).
                nc.sync.dma_start(
                    xbo_s[:], xbo.ap().rearrange("t p m -> p t m")
                )
                wv_h = [load_w_half(wvt, 0, "v", dt=bf16, tag="wv"),
                        load_w_half(wvt, 1, "v", dt=bf16, tag="wv")]
                nc.sync.dma_start(
                    cc2_s[:], cc2.ap().rearrange("t p c -> p t c")
                )
                nc.sync.dma_start(
                    wra_s[:], wra.ap().rearrange("t p c -> p t c")
                )
                m1_h = [load_w_half(m1t, h, "m1", dt=bf16, tag="wb", bufs=4)
                        for h in range(2)]
                m2_h = [load_w_half(m2t, h, "m2", dt=bf16, tag="wb", bufs=4)
                        for h in range(2)]
                # per d-half: project, bias, store, gather — chunked so the
                # dh=0 gather is in flight while dh=1 is still projecting.
                # Stores + collectives ride the gpsimd queue so they never
                # block the SP weight-load stream.
                for dh in range(2):
                    for mt in range(MT):
                        vps = ps1.tile(
                            [128, 512], f32, tag="ps1", name=f"vps{dh}{mt}"
                        )
                        for t in range(DC):
                            nc.tensor.matmul(
                                vps[:],
                                xbo_s[:, t, mt * 128 : (mt + 1) * 128],
                                wv_h[dh][:, t, :],
                                start=(t == 0),
                                stop=False,
                            )
                        # high priority: the gathers (and thus the whole
                        # collective pipeline) hang off the stores
                        with tc.high_priority():
                            nc.tensor.matmul(
                                vps[:],
                                onesm_s[:, :],
                                bv_s[:, dh * 512 : (dh + 1) * 512],
                                start=False,
                                stop=True,
                            )
                            v_sb = sp.tile([128, 512], bf16, tag="vsb",
                                           name="v_sb", bufs=8)
                            # DVE copy: keeps the chain off the ACT queue's
                            # table-load startup
                            nc.vector.tensor_scalar(
                                v_sb[:], vps[:], 1.0, None, ALU.mult
                            )
                            nc.gpsimd.dma_start(v_loc[dh][mt], v_sb[:])
                    if not timing_mode:
                        nc.gpsimd.collective_compute(
                            "AllGather", ALU.bypass, replica_groups=RG,
                            ins=[v_loc[dh][:].opt()], outs=[zv[dh][:].opt()],
                        )

                # rowA = x_o @ wrA + sA  (one [1,512] strip)
                rps = ps1.tile([1, M], f32, tag="ps1", name="rps")
                for t in range(DC):
                    nc.tensor.matmul(
                        rps[:],
                        wra_s[:, t, :],
                        xbo_s[:, t, :],
                        start=(t == 0),
                        stop=(t == DC - 1),
                    )
                nc.scalar.activation(
                    rowa_s[:], rps[:], AF.Identity, bias=sa_s[:, 0:1]
                )

                # PB = M1^T-chunks @ x_o^T ; PA = M2^T-chunks @ x_o^T
                def fold_proj(m_h, dst):
                    for half in range(2):
                        fps = [
                            ps1.tile([128, M], f32, tag="ps1", name="fps")
                            for _ in range(4)
                        ]
                        for t in range(DC):
                            for oi in range(4):
                                nc.tensor.matmul(
                                    fps[oi][:],
                                    m_h[half][:, t, oi * 128 : (oi + 1) * 128],
                                    xbo_s[:, t, :],
                                    start=(t == 0),
                                    stop=(t == DC - 1),
                                )
                        for oi in range(4):
                            ot = half * 4 + oi
                            nc.scalar.copy(dst[:, ot, :], fps[oi][:])

                fold_proj(m1_h, pb_s)
                fold_proj(m2_h, pa_s)

            # v tiles + xn survive into the O phase
            o_stack = contextlib.ExitStack()
            vpool = o_stack.enter_context(
                tc.tile_pool(name="o_v", bufs=3, side="right")
            )
            xop = o_stack.enter_context(
                tc.tile_pool(name="o_x", bufs=1, side="right")
            )
            xn_s = xop.tile([128, MT, D], f32, name="xn_s")
            vt_pre = {}

            def load_vt(dh, j):
                vt = vpool.tile([128, 4, 512], bf16, tag="v", name="vt", bufs=4)
                for vb in range(4):
                    # gpsimd queue: these loads wait on the collective, and on
                    # the in-order SP queue they would head-of-line-block the
                    # S phase's mask/xtb streams
                    nc.gpsimd.dma_start(vt[:, vb, :], zv[dh][j][vb])
                vt_pre[(dh, j)] = vt
                return vt

            # ------- S phase: logits via folded mats, mask, exp -------
            with (
                tc.tile_pool(name="s_x", bufs=3) as xbp,
                tc.tile_pool(name="s_kc", bufs=3) as kcp,
                tc.tile_pool(name="s_m", bufs=6) as mp,
                tc.tile_pool(name="s_t", bufs=4) as tpool,
                tc.tile_pool(name="s_psK", bufs=2, space="PSUM") as psK,
                tc.tile_pool(name="s_psA", bufs=3, space="PSUM") as psA,
                tc.tile_pool(name="s_psB", bufs=3, space="PSUM") as psB,
            ):
                xtb_pre = {}

                def load_xtb(j, eng=None):
                    xtb = xbp.tile([128, DC, M], bf16, tag="xtb", name="xtb")
                    (eng or nc.sync).dma_start(
                        xtb[:], xtf.ap()[j].rearrange("t p m -> p t m")
                    )
                    xtb_pre[j] = xtb
                    return xtb

                kc_pre = {}
                mk_pre = {}

                def load_mk(g):
                    mk_t = mp.tile([128, 2 * M], mybir.dt.uint8, tag="mk",
                                   name="mk_t")
                    nc.sync.dma_start(mk_t[:], mk.ap()[g])
                    mk_pre[g] = mk_t
                    return mk_t

                for g0 in range(4):
                    load_mk(g0)

                def compute_kc(j, xtb):
                    # separate [1, M] groups: engines cannot address a
                    # partition range starting at 1, so a [2, M] strip's
                    # second row would be unreadable
                    outs = []
                    for ci, tag in ((0, "kcb"), (1, "kca")):
                        kps = psK.tile([1, M], f32, tag="kc", name=f"kcps{ci}")
                        for t in range(DC):
                            nc.tensor.matmul(
                                kps[:],
                                cc2_s[:, t, ci : ci + 1],
                                xtb[:, t, :],
                                start=(t == 0),
                                stop=(t == DC - 1),
                            )
                        kc = kcp.tile([1, M], bf16, tag=tag, name=tag)
                        nc.scalar.copy(kc[:], kps[:])
                        outs.append(kc)
                    kc_pre[j] = tuple(outs)

                load_xtb(0)
                for j in range(NCORES):
                    xtb = xtb_pre.pop(j)
                    if j + 1 < NCORES:
                        # ACT queue: dispatch is paced by the S phase's exp
                        # stream, so these 2MB transfers stay out of the DMA
                        # window the v stores + gathers need
                        load_xtb(j + 1, eng=nc.scalar)
                    compute_kc(j, xtb)
                    kcb, kca = kc_pre.pop(j)
                    for gi in range(4):
                        g = j * 4 + gi
                        B = psB.tile([128, M], f32, tag="B", name="Bps")
                        for t in range(DC):
                            nc.tensor.matmul(
                                B[:],
                                xtb[:, t, gi * 128 : (gi + 1) * 128],
                                pb_s[:, t, :],
                                start=(t == 0),
                                stop=False,
                            )
                        nc.tensor.matmul(
                            B[:], kcb[:, gi * 128 : (gi + 1) * 128],
                            onesb_s[:], start=False, stop=True,
                        )
                        A = psA.tile([128, M], f32, tag="A", name="Aps")
                        for t in range(DC):
                            nc.tensor.matmul(
                                A[:],
                                xtb[:, t, gi * 128 : (gi + 1) * 128],
                                pa_s[:, t, :],
                                start=(t == 0),
                                stop=False,
                            )
                        nc.tensor.matmul(
                            A[:], kca[:, gi * 128 : (gi + 1) * 128],
                            onesb_s[:], start=False, stop=False,
                        )
                        nc.tensor.matmul(
                            A[:], onesm_s[:], rowa_s[:], start=False, stop=True,
                        )
                        if g + 4 < G:
                            load_mk(g + 4)
                        mk_t = mk_pre.pop(g)
                        t3 = tpool.tile([128, M], f32, tag="t3", name="t3")
                        nc.vector.scalar_tensor_tensor(
                            t3[:], A[:], -bias_val, mk_t[:, M : 2 * M],
                            ALU.is_gt, ALU.mult,
                        )
                        nc.vector.tensor_tensor(
                            t3[:], t3[:], mk_t[:, 0:M], ALU.add
                        )
                        comb = tpool.tile([128, M], f32, tag="comb", name="comb")
                        nc.vector.scalar_tensor_tensor(
                            comb[:], t3[:], MSCALE, B[:], ALU.mult, ALU.add
                        )
                        # -20000 = the (am+st*lm-2) shift, folded into the exp bias
                        nc.scalar.activation(
                            E3[g][:], comb[:], AF.Exp, scale=1.0 / 32.0,
                            bias=shift_s[:, 0:1],
                        )
                    if j == 1:
                        # xn is consumed only by the O phase; emitting the
                        # load here keeps its 2MB transfer out of the early
                        # DMA window that the v stores + gathers need
                        nc.sync.dma_start(
                            xn_s[:], xn.ap().rearrange("m p d -> p m d")
                        )
            q_stack.close()  # PB/PA/xbo SBUF released before O phase

            # ---------------- O phase: E @ v, sums, blend ----------------
            with (
                tc.tile_pool(name="o_out", bufs=4) as opool,
                tc.tile_pool(name="o_xm", bufs=1) as xmp,
                tc.tile_pool(name="o_ps", bufs=1, space="PSUM") as psO,
            ):
                # xm = (1-boundary)*x for all 8 chunks, off the critical path
                xm_t = {}
                for mt in range(MT):
                    for dh in range(2):
                        xm = xmp.tile([128, 512], f32, name=f"xm_{mt}_{dh}")
                        nc.vector.tensor_scalar(
                            xm[:],
                            xn_s[:, mt, dh * 512 : (dh + 1) * 512],
                            omb_s[:, mt : mt + 1],
                            None,
                            ALU.mult,
                        )
                        xm_t[(mt, dh)] = xm
                # Ssum first: needs no v, so it runs while gathers are still
                # in flight and buys the dh=1 gather a little more slack
                S_ps = [
                    psO.tile([128, 8], f32, tag="Ssum", name=f"S{mt}", bufs=4)
                    for mt in range(MT)
                ]
                for g in range(G):
                    for mt in range(MT):
                        nc.tensor.matmul(
                            S_ps[mt][:],
                            E3[g][:, mt * 128 : (mt + 1) * 128],
                            ones_s[:],
                            start=(g == 0),
                            stop=(g == G - 1),
                        )
                for mt in range(MT):
                    nc.vector.reciprocal(
                        recip_s[:, mt : mt + 1], S_ps[mt][:, 0:1]
                    )
                    nc.vector.tensor_tensor(
                        s1_s[:, mt : mt + 1],
                        recip_s[:, mt : mt + 1],
                        bnd_s[:, mt : mt + 1],
                        ALU.mult,
                    )
                for dh in range(2):
                    O_ps = [
                        psO.tile([128, 512], f32, tag="O", name=f"O{dh}_{mt}", bufs=4)
                        for mt in range(MT)
                    ]
                    for j in range(NCORES):
                        vt = vt_pre.pop((dh, j), None) or load_vt(dh, j)
                        vt_pre.pop((dh, j), None)
                        for gi in range(4):
                            g = j * 4 + gi
                            for mt in range(MT):
                                nc.tensor.matmul(
                                    O_ps[mt][:],
                                    E3[g][:, mt * 128 : (mt + 1) * 128],
                                    vt[:, gi, :],
                                    start=(g == 0),
                                    stop=(g == G - 1),
                                )
                    for mt in range(MT):
                        # out = O*(boundary/sumexp) + (1-boundary)*x, one DVE op
                        ot_t = opool.tile([128, 512], f32, tag="ot", name="ot_t")
                        nc.vector.scalar_tensor_tensor(
                            ot_t[:], O_ps[mt][:], s1_s[:, mt : mt + 1],
                            xm_t[(mt, dh)][:], ALU.mult, ALU.add,
                        )
                        nc.sync.dma_start(
                            out.ap()[mt, :, dh * 512 : (dh + 1) * 512], ot_t[:]
                        )
            o_stack.close()

    nc.compile()
    return nc


def make_in_maps(x, attention_mask, learnable_mask, boundary_mask,
                 W_q, b_q, W_k, b_k, W_v, b_v, connection):
    bf = ml_dtypes.bfloat16
    x = np.asarray(x, np.float32)
    W_q = np.asarray(W_q, np.float32)
    W_k = np.asarray(W_k, np.float32)
    W_v = np.asarray(W_v, np.float32)
    C = np.asarray(connection, np.float32)
    b_q = np.asarray(b_q, np.float32)
    b_k = np.asarray(b_k, np.float32)
    amh_full = np.asarray(attention_mask, np.float32).astype(np.uint8)
    lmh_full = np.asarray(learnable_mask, np.float32).astype(np.uint8)
    boundary = np.asarray(boundary_mask, np.float32).reshape(N)

    # Folded matrices (f32 accumulate, bf16 ship)
    M1 = W_k.T @ W_q                       # B main term
    M2 = W_k.T @ C.T @ W_q                 # A main term
    c1 = W_k.T @ b_q                       # B per-k-row column
    c2 = W_k.T @ (C.T @ b_q)               # A per-k-row column
    wrA = (C @ b_k) @ W_q                  # A per-own-row vector
    sA = float(b_q @ (C @ b_k))            # A scalar

    xtf_h = np.ascontiguousarray(
        np.stack([x[j * M : (j + 1) * M].T.astype(bf) for j in range(NCORES)])
    ).reshape(NCORES, DC, 128, M)
    m1t_h = np.ascontiguousarray(M1.T.astype(bf)).reshape(DC, 128, D)
    m2t_h = np.ascontiguousarray(M2.T.astype(bf)).reshape(DC, 128, D)
    cc2_h = np.ascontiguousarray(
        np.stack([c1, c2], axis=1).astype(bf)
    ).reshape(DC, 128, 2)
    wra_h = np.ascontiguousarray(wrA.astype(bf)).reshape(DC, 128, 1)
    wvt_h = np.ascontiguousarray(W_v.T.astype(bf)).reshape(DC, 128, D)
    bv_h = np.ascontiguousarray(np.asarray(b_v, np.float32).astype(bf).reshape(1, D))
    in_maps = []
    for c in range(NCORES):
        rows = slice(c * M, (c + 1) * M)
        # packed masks: per g-tile [128 k-rows, own 512 | own 512]
        amT = np.ascontiguousarray(amh_full[rows].T).reshape(G, 128, M)
        lmT = np.ascontiguousarray(lmh_full[rows].T).reshape(G, 128, M)
        mk_h = np.ascontiguousarray(np.concatenate([amT, lmT], axis=2))
        in_maps.append(dict(
            xn=np.ascontiguousarray(x[rows]).reshape(MT, 128, D),
            xbo=np.ascontiguousarray(x[rows].T.astype(bf)).reshape(DC, 128, M),
            xtf=xtf_h,
            m1t=m1t_h, m2t=m2t_h, cc2=cc2_h, wra=wra_h,
            sa1=np.full((1, 1), sA, np.float32),
            wvt=wvt_h, bvb=bv_h,
            bnd=np.ascontiguousarray(boundary[rows]).reshape(MT, 128, 1),
            mk=mk_h,
            ones8=np.ones((128, 8), dtype=bf),
            ones1=np.ones((1, 128), dtype=np.float32),
            onesb=np.ones((1, M), dtype=bf),
        ))
    return in_maps


_cache = {}


def kernel(x, attention_mask, learnable_mask, boundary_mask,
           W_q, b_q, W_k, b_k, W_v, b_v, connection, bias):
    bias_val = float(np.asarray(bias).reshape(-1)[0])
    if bias_val not in _cache:
        _cache[bias_val] = build(bias_val)
    nc = _cache[bias_val]
    in_maps = make_in_maps(x, attention_mask, learnable_mask, boundary_mask,
                           W_q, b_q, W_k, b_k, W_v, b_v, connection)
    res = bass_utils.run_bass_kernel_spmd(nc, in_maps, core_ids=list(range(NCORES)))
    outs = [res.results[c]["out"].reshape(M, D) for c in range(NCORES)]
    return np.concatenate(outs, axis=0).astype(np.float32)
